# revision 21
# baseline (speedup 1.0000x reference)
"""Single-head attention on 8 Trainium2 NeuronCores.

Problem: x[8, 2048, 768], Wq/Wk/Wv[768, 64]+biases, mask[2048, 2048] int32
Output:  softmax(mask(Q K^T / 8)) V   -> [8, 2048, 64] f32

Sharding: data-parallel over batch — core b computes batch element b.

Per-core dataflow (all matmuls bf16 in / fp32 psum accumulate):
  host:  xT = x[b].T (w-major, partition-major relayout), Wqk = [Wq | Wk/8],
         mT = mask.T (k-major, 0/1 bf16, partition-major relayout)
  QK:    QK[n,128] = xT.T @ Wqk + bqk (bias via K=1 matmul), cast bf16 ->
         QQ/KK duplicated across partition halves (row-tiled score matmuls)
  V:     V[n,64] = xT.T @ Wv + bv, stored as V'[k,65] with ones column so the
         PV matmul also produces the softmax denominator for free
  ST:    ST[k,q] = KK.T @ QQ per 128-k-chunk (K=64 contraction: two chunks run
         concurrently in the PE array via row tiling at partitions 0/64)
  P:     P = exp(ST) on ScalarE (psum -> sbuf bf16), P *= mT (VectorE)
  OT:    OT[65,q] += V'[kchunk].T @ P[kchunk]  (accumulate over 16 k-chunks;
         k is the OUTER loop so each mask chunk is consumed for both q tiles
         right when it lands — mask DMAs are chained behind xT so chunks
         arrive in consumption order at full bandwidth)
  out:   OT[65,2048] copied psum->sbuf and DMA'd; host does the final
         out[q,h] = OT[h,q]/OT[64,q] normalization + transpose (gather step).
"""

import numpy as np
import ml_dtypes

import bass_rust
import concourse.bass as bass
import concourse.mybir as mybir
import concourse.tile as tile
from concourse.bass_utils import run_bass_kernel_spmd

BF16 = ml_dtypes.bfloat16
F32 = mybir.dt.float32
BF = mybir.dt.bfloat16

N_CORES = 8
SEQ = 2048
WIDTH = 768
HEAD = 64
NCH = WIDTH // 128      # 6 contraction chunks for the projections
NKC = SEQ // 128        # 16 key chunks
QT = 1024               # q tile (columns processed per main-loop sweep)
NQT = SEQ // QT


def _split_excess_waits(nc, max_waits=1):
    """walrus in this container rejects >1 sync wait per instruction; hoist
    extras onto preceding same-engine NoOps (same semantics: the engine
    executes its stream in order, so waiting earlier is equivalent)."""
    n = 0
    for bb in nc.main_func.blocks:
        new_list = []
        for ins in bb.instructions:
            si = ins.sync_info
            if si is not None and len(si.on_wait) > max_waits:
                waits = list(si.on_wait)
                extra, keep = waits[:-max_waits], waits[-max_waits:]
                for j, w in enumerate(extra):
                    nop = bass_rust.InstNoOp(
                        name=f"{ins.name}-ws{j}", engine=ins.engine, ins=[], outs=[]
                    )
                    nop.sync_info = mybir.SyncInfo(on_wait=[w], on_update=[])
                    new_list.append(nop)
                    n += 1
                ins.sync_info = mybir.SyncInfo(
                    on_wait=keep, on_update=list(si.on_update)
                )
            new_list.append(ins)
        bb.instructions = new_list
    return n


def _strip_tail(nc):
    """Drop the NRT pseudo-sync ISA op and the second all-engine barrier that
    TileContext emits after the semaphore reset — ~4-5us of fixed tail. The
    final DMA-drain + first barrier + sem reset are kept, so re-execution of
    the NEFF still starts from clean semaphores."""
    for bb in nc.main_func.blocks:
        ins_list = list(bb.instructions)
        idx = None
        for i, ins in enumerate(ins_list):
            if getattr(ins, "is_reset_sema", False):
                idx = i
        if idx is not None and idx > len(ins_list) - 20:
            bb.instructions = ins_list[:idx + 1]
    return nc


def _build():
    nc = bass.Bass("TRN2", target_bir_lowering=False, debug=False,
                   num_devices=N_CORES)

    # partition-major host layouts: row p holds everything partition p needs,
    # so each DMA is 128 large contiguous descriptors.
    xT_d = nc.declare_dram_parameter("xT", [128, 4 * NCH * 512], BF, False).ap()
    wqk_d = nc.declare_dram_parameter("Wqk", [128, NCH * 128], BF, False).ap()
    wv_d = nc.declare_dram_parameter("Wv", [128, NCH * HEAD], BF, False).ap()
    bqk_d = nc.declare_dram_parameter("bqk", [128, 1], F32, False).ap()
    bv_d = nc.declare_dram_parameter("bv", [1, HEAD], BF, False).ap()
    mT_d = nc.declare_dram_parameter("mT", [128, NKC * SEQ], BF, False).ap()
    ot_d = nc.declare_dram_parameter("ot", [HEAD + 1, SEQ], F32, True).ap()

    EXP = mybir.ActivationFunctionType.Exp
    COPY = mybir.ActivationFunctionType.Copy

    with tile.TileContext(nc) as tc:
        with (
            tc.tile_pool(name="const", bufs=1) as const,
            tc.tile_pool(name="pp", bufs=6) as ppool,
            tc.tile_pool(name="ep", bufs=2) as epool,
            tc.tile_pool(name="stp", bufs=3, space="PSUM") as stp,
            tc.tile_pool(name="otp", bufs=1, space="PSUM") as otp,
        ):
            # ---- inputs into SBUF (all on the SP hardware-DGE path) ----
            wqk = const.tile([128, NCH, 128], BF)
            nc.sync.dma_start(out=wqk, in_=wqk_d)
            wv = const.tile([128, NCH, HEAD], BF)
            nc.sync.dma_start(out=wv, in_=wv_d)
            bqk = const.tile([128, 1], F32)
            nc.sync.dma_start(out=bqk, in_=bqk_d)
            bv = const.tile([1, HEAD], BF)
            nc.sync.dma_start(out=bv, in_=bv_d)
            xt = const.tile([128, 4, NCH, 512], BF)
            for t in range(4):
                nc.sync.dma_start(
                    out=xt[:, t, :, :],
                    in_=xT_d[:, t * NCH * 512:(t + 1) * NCH * 512],
                )
            ones = const.tile([1, 512], BF)
            nc.vector.memset(ones, 1.0)

            # mask halves in consumption order (all q-tile-0 halves first,
            # chunk-major), emitted after xT so the descriptor ring drains
            # them behind xT — chunks then land just ahead of their use.
            mt = const.tile([128, NKC, SEQ], BF)
            for h in range(NQT):
                for c in range(NKC):
                    src = (h * NKC + c) * QT
                    nc.sync.dma_start(
                        out=mt[:, c, h * QT:(h + 1) * QT],
                        in_=mT_d[:, src:src + QT],
                    )

            # ---- projections, one 512-column group per xT DMA so they
            # pipeline behind the xT arrival; groups t2/t3 are emitted
            # interleaved into the early main loop (their xT lands later) ----
            qktmp = const.tile([128, SEQ], BF)   # Q on parts 0:64, K on 64:128
            qq = const.tile([128, SEQ], BF)      # Q duplicated on both halves
            kk = const.tile([128, SEQ], BF)      # K duplicated on both halves
            vp = const.tile([128, NKC, HEAD + 1], BF)   # V' with ones column

            def proj_qk(t):
                qk_ps = stp.tile([128, 512], F32, tag="st", name=f"qk_ps{t}")
                for c in range(NCH):
                    nc.tensor.matmul(
                        qk_ps, wqk[:, c, :], xt[:, t, c, :],
                        start=(c == 0), stop=(c == NCH - 1),
                    )
                cols = slice(t * 512, (t + 1) * 512)
                nc.vector.tensor_scalar(
                    out=qktmp[:, cols], in0=qk_ps, scalar1=bqk[:, 0:1],
                    scalar2=None, op0=mybir.AluOpType.add,
                )
                nc.vector.tensor_copy(out=qq[0:64, cols], in_=qktmp[0:64, cols])
                nc.vector.tensor_copy(out=qq[64:128, cols], in_=qktmp[0:64, cols])
                nc.vector.tensor_copy(out=kk[0:64, cols], in_=qktmp[64:128, cols])
                nc.vector.tensor_copy(out=kk[64:128, cols], in_=qktmp[64:128, cols])

            def proj_v(t, jlo, jhi):
                v_ps = stp.tile([128, jhi - jlo, HEAD], F32, tag="st",
                                name=f"v_ps{t}_{jlo}")
                for j in range(jlo, jhi):
                    for c in range(NCH):
                        nc.tensor.matmul(
                            v_ps[:, j - jlo, :],
                            xt[:, t, c, j * 128:(j + 1) * 128],
                            wv[:, c, :], start=(c == 0), stop=False,
                        )
                    nc.tensor.matmul(   # +bias: ones[m] * bv[n]
                        v_ps[:, j - jlo, :], ones[0:1, 0:128], bv[0:1, :],
                        start=False, stop=True,
                    )
                lo, hi = 4 * t + jlo, 4 * t + jhi
                nc.vector.tensor_copy(out=vp[:, lo:hi, 0:HEAD], in_=v_ps)
                nc.vector.memset(vp[:, lo:hi, HEAD:HEAD + 1], 1.0)

            proj_qk(0)
            proj_v(0, 0, 4)
            proj_qk(1)

            # ---- main loop (q outer): scores -> exp -> mask -> PV ----
            # PV for iteration kp-1 is emitted alongside the scores for kp
            # (as a list of matmul specs) so the PE never stalls mid-stream.
            # The first two iterations of q0 are split into 512-wide halves:
            # the h0 half depends only on xT group t0, so exp starts ~4us
            # earlier while t1 is still arriving.
            NKP = NKC // 2
            for q in range(NQT):
                ot_ps = otp.tile([HEAD + 1, QT], F32, tag="ot", name=f"ot_ps{q}")
                qc = slice(q * QT, (q + 1) * QT)
                prev = []
                for kp in range(NKP + 1):
                    cur = []
                    if kp < NKP:
                        k0, k1 = 2 * kp, 2 * kp + 1
                        split = (q == 0 and kp < 2)
                        halves = ((0, 512), (512, 1024)) if split else ((0, QT),)
                        for lo, hi in halves:
                            w = hi - lo
                            st_a = stp.tile([128, w], F32, tag="st",
                                            name=f"st_a{q}_{kp}_{lo}")
                            st_b = stp.tile([128, w], F32, tag="st",
                                            name=f"st_b{q}_{kp}_{lo}")
                            for h in range(w // 512):
                                gq = slice(q * QT + lo + h * 512,
                                           q * QT + lo + (h + 1) * 512)
                                nc.tensor.matmul(
                                    st_a[:, h * 512:(h + 1) * 512],
                                    kk[0:64, k0 * 128:(k0 + 1) * 128],
                                    qq[0:64, gq], start=True, stop=True,
                                )
                                nc.tensor.matmul(
                                    st_b[:, h * 512:(h + 1) * 512],
                                    kk[64:128, k1 * 128:(k1 + 1) * 128],
                                    qq[64:128, gq], start=True, stop=True,
                                )
                            p_a = ppool.tile([128, w], BF, tag="p",
                                             name=f"p_a{q}_{kp}_{lo}")
                            p_b = ppool.tile([128, w], BF, tag="p",
                                             name=f"p_b{q}_{kp}_{lo}")
                            nc.scalar.activation(p_a, st_a, EXP)
                            nc.scalar.activation(p_b, st_b, EXP)
                            gqs = slice(q * QT + lo, q * QT + hi)
                            nc.vector.tensor_mul(p_a, p_a, mt[:, k0, gqs])
                            nc.vector.tensor_mul(p_b, p_b, mt[:, k1, gqs])
                            for h in range(w // 512):
                                ohs = slice(lo + h * 512, lo + (h + 1) * 512)
                                phs = slice(h * 512, (h + 1) * 512)
                                cur.append((ohs, k0, p_a, phs))
                                cur.append((ohs, k1, p_b, phs))
                        if q == 0:
                            # feed remaining projection work into the PE
                            # stream in small wedges so ScalarE never starves
                            if kp == 0:
                                proj_v(1, 0, 2)
                            elif kp == 1:
                                proj_v(1, 2, 4)
                                proj_qk(2)
                            elif kp == 2:
                                proj_v(2, 0, 2)
                            elif kp == 3:
                                proj_v(2, 2, 4)
                                proj_qk(3)
                            elif kp == 4:
                                proj_v(3, 0, 2)
                            elif kp == 5:
                                proj_v(3, 2, 4)
                    for ohs, k, p, phs in prev:
                        nc.tensor.matmul(
                            ot_ps[:, ohs], vp[:, k, :], p[:, phs],
                            start=(k == 0), stop=(k == NKC - 1),
                        )
                    prev = cur

                # psum -> sbuf, then DMA out. q0's copy runs mid-loop where
                # ScalarE is saturated, so it goes on VectorE; q1's copy runs
                # at the tail where ScalarE is free.
                ot_sb = epool.tile([HEAD + 1, QT], F32, tag="osb", name=f"ot_sb{q}")
                if q == 0:
                    nc.vector.tensor_copy(out=ot_sb, in_=ot_ps)
                else:
                    nc.scalar.activation(ot_sb, ot_ps, COPY)
                nc.sync.dma_start(out=ot_d[:, q * QT:(q + 1) * QT], in_=ot_sb)

    _split_excess_waits(nc)
    _strip_tail(nc)
    return nc


_CACHE = {}


def _get_nc():
    if "nc" not in _CACHE:
        _CACHE["nc"] = _build()
    return _CACHE["nc"]


def _prep_in_maps(x, Wq, bq, Wk, bk, Wv, bv, mask):
    x = np.asarray(x, dtype=np.float32)
    Wqk = np.concatenate(
        [np.asarray(Wq, np.float32), np.asarray(Wk, np.float32) * 0.125], axis=1
    )
    # partition-major: row p holds [c0 cols | c1 cols | ...] for w = c*128+p
    Wqkh = np.ascontiguousarray(
        Wqk.reshape(NCH, 128, 128).transpose(1, 0, 2).reshape(128, NCH * 128)
    ).astype(BF16)
    Wvh = np.ascontiguousarray(
        np.asarray(Wv, np.float32).reshape(NCH, 128, HEAD)
        .transpose(1, 0, 2).reshape(128, NCH * HEAD)
    ).astype(BF16)
    bqk = np.concatenate(
        [np.asarray(bq, np.float32), np.asarray(bk, np.float32) * 0.125]
    ).astype(np.float32).reshape(128, 1)
    bv16 = np.asarray(bv, np.float32).astype(BF16).reshape(1, HEAD)
    # mTh[p, (h*NKC+c)*QT + j] = mask[h*QT+j, c*128+p]: all q-half-0 chunk
    # slices first, then q-half-1 — matches the DMA emission order.
    mTh = np.ascontiguousarray(
        np.asarray(mask, np.float32).T.reshape(NKC, 128, NQT, QT)
        .transpose(1, 2, 0, 3).reshape(128, NKC * SEQ)
    ).astype(BF16)
    in_maps = []
    for b in range(N_CORES):
        # xth[p, t, c, j] = x[b][t*512+j, c*128+p]
        xth = np.ascontiguousarray(
            x[b].reshape(4, 512, NCH, 128).transpose(3, 0, 2, 1)
            .reshape(128, 4 * NCH * 512)
        ).astype(BF16)
        in_maps.append({
            "xT": xth, "Wqk": Wqkh, "Wv": Wvh, "bqk": bqk, "bv": bv16,
            "mT": mTh,
        })
    return in_maps


def _run(in_maps, trace=False, **kw):
    nc = _get_nc()
    return run_bass_kernel_spmd(nc, in_maps, list(range(N_CORES)), trace=trace, **kw)


def kernel(x, Wq, bq, Wk, bk, Wv, bv, mask):
    in_maps = _prep_in_maps(x, Wq, bq, Wk, bk, Wv, bv, mask)
    res = _run(in_maps)
    out = np.empty((N_CORES, SEQ, HEAD), np.float32)
    for b in range(N_CORES):
        ot = np.asarray(res.results[b]["ot"])          # [65, 2048] f32
        out[b] = (ot[:HEAD] / ot[HEAD:HEAD + 1]).T     # normalize + transpose
    return out
